# revision 17
# baseline (speedup 1.0000x reference)
"""Multi-head causal self-attention (32 heads, RoPE) on 8 Trainium2 cores.

Tensor-parallel over heads: core c owns heads 4c..4c+3 (512 of 4096 qkv dims).
Each core computes q/k/v projections for its heads, RoPE, causal softmax
attention fused with its partial o-projection (per query chunk, straight from
SBUF); the 8 partials are reduce-scattered on device.

Data movement: shared inputs (xT, RoPE tables, mask) are shipped to the
device once, column-sharded, and replicated on-device via all_gather; the
partial outputs are combined by an on-device psum_scatter + transpose so only
the final [rows, hs] f32 result crosses the (slow) axon tunnel.

Layouts (per core):
  xT    [4096 hs, 4096 rows]  bf16   rows = b*2048 + t
  qT/kT [512 d, 4096 rows]    bf16   (transposed: head dim on partitions)
  v     [4096 rows, 512 d]    bf16   (row-major)
  out   [4096 cols, 4096 rows] f32   partial of (attn_out @ wo)^T

Softmax runs on transposed scores sT[j,i] (keys on partitions): no-max-sub
exp (scores ~N(0,1)), column sums via ones-matmul on the PE, late
normalization with a partition-broadcast reciprocal.
"""
import sys

for _p in ("/opt/trn_rl_repo", "/root/.axon_site/_ro/trn_rl_repo"):
    if _p not in sys.path:
        sys.path.append(_p)

import numpy as np
import ml_dtypes

import concourse.bacc as bacc
import concourse.mybir as mybir
import concourse.tile as tile

BF16 = mybir.dt.bfloat16
F32 = mybir.dt.float32
BFNP = ml_dtypes.bfloat16

N_CORES = 8
BS, SL, HS = 2, 2048, 4096
NH, HD = 32, 128
HPC = NH // N_CORES          # heads per core = 4
DPC = HPC * HD               # qkv dims per core = 512
ROWS = BS * SL               # 4096
P = 128
MC = 512                     # m-chunk (rows) width
NMC = ROWS // MC             # 8 m-chunks
NKT = HS // P                # 32 contraction tiles
NIC = SL // MC               # 4 query chunks per sequence
NJT = SL // P                # 16 key tiles per sequence
SCALE = float(HD) ** -0.5
ROPE_THETA = 10000.0

ExpF = mybir.ActivationFunctionType.Exp
CopyF = mybir.ActivationFunctionType.Copy


def build_program():
    nc = bacc.Bacc("TRN2", target_bir_lowering=False, debug=False,
                   num_devices=N_CORES)

    xT_d = nc.dram_tensor("xT", [HS, ROWS], BF16, kind="ExternalInput").ap()
    wq_d = nc.dram_tensor("wq", [HS, DPC], BF16, kind="ExternalInput").ap()
    wk_d = nc.dram_tensor("wk", [HS, DPC], BF16, kind="ExternalInput").ap()
    wv_d = nc.dram_tensor("wv", [HS, DPC], BF16, kind="ExternalInput").ap()
    wo_d = nc.dram_tensor("wo", [DPC, HS], BF16, kind="ExternalInput").ap()
    cosq_d = nc.dram_tensor("cosq", [P, ROWS], F32, kind="ExternalInput").ap()
    sinq_d = nc.dram_tensor("sinq", [P, ROWS], F32, kind="ExternalInput").ap()
    cosk_d = nc.dram_tensor("cosk", [P, ROWS], F32, kind="ExternalInput").ap()
    sink_d = nc.dram_tensor("sink", [P, ROWS], F32, kind="ExternalInput").ap()
    mask_d = nc.dram_tensor("mask", [P, 4 * MC], BF16, kind="ExternalInput").ap()
    out_d = nc.dram_tensor("out", [HS, ROWS], F32, kind="ExternalOutput").ap()

    qT_d = nc.dram_tensor("qT_i", [DPC, ROWS], BF16).ap()
    kT_d = nc.dram_tensor("kT_i", [DPC, ROWS], BF16).ap()
    v_d = nc.dram_tensor("v_i", [ROWS, DPC], BF16).ap()

    with tile.TileContext(nc) as tc:
        with tc.tile_pool(name="const", bufs=1) as const_pool:
            ones_sb = const_pool.tile([P, P], BF16, tag="ones")
            nc.vector.memset(ones_sb[:], 1.0)

            # ---------------- Phase 1: q/k/v projections + RoPE ----------
            with (
                tc.tile_pool(name="wqk", bufs=1) as wqk_pool,
                tc.tile_pool(name="xb", bufs=2) as x_pool,
                tc.tile_pool(name="wvt", bufs=3) as wv_pool,
                tc.tile_pool(name="trig", bufs=2) as trig_pool,
                tc.tile_pool(name="rope", bufs=3) as rope_pool,
                tc.tile_pool(name="qko", bufs=4) as qko_pool,
                tc.tile_pool(name="vo", bufs=3) as vo_pool,
                tc.tile_pool(name="psv", bufs=1, space="PSUM") as ps_v,
                tc.tile_pool(name="psqk", bufs=2, space="PSUM") as ps_qk,
            ):
                wq_sb = wqk_pool.tile([P, NKT * DPC], BF16, tag="wq")
                wk_sb = wqk_pool.tile([P, NKT * DPC], BF16, tag="wk")
                xblk0 = x_pool.tile([P, NKT * MC], BF16, tag="xblk")
                # chunk-0 x gets the DMA engines to itself (wq/wk gated on
                # its completion) so the v-matmul chain starts ~13us in;
                # wq/wk then stream under the chunk-0 compute.
                xb0_dma = nc.sync.dma_start(
                    xblk0[:].rearrange("p (k m) -> p k m", k=NKT),
                    xT_d[:, 0:MC].rearrange("(k p) m -> p k m", p=P),
                )
                wq_dma = nc.sync.dma_start(
                    wq_sb[:].rearrange("p (k n) -> p k n", k=NKT),
                    wq_d.rearrange("(k p) n -> p k n", p=P),
                )
                wk_dma = nc.sync.dma_start(
                    wk_sb[:].rearrange("p (k n) -> p k n", k=NKT),
                    wk_d.rearrange("(k p) n -> p k n", p=P),
                )
                from concourse.tile import add_dep_helper
                for dep in (wq_dma, wk_dma):
                    add_dep_helper(
                        getattr(dep, "ins", dep), getattr(xb0_dma, "ins", xb0_dma),
                        reason="let chunk-0 x win the DMA bandwidth race")

                for mc in range(NMC):
                    ms = mc * MC
                    if mc == 0:
                        xblk = xblk0
                    else:
                        xblk = x_pool.tile([P, NKT * MC], BF16, tag="xblk")
                        nc.sync.dma_start(
                            xblk[:].rearrange("p (k m) -> p k m", k=NKT),
                            xT_d[:, ms:ms + MC].rearrange("(k p) m -> p k m", p=P),
                        )
                    # --- v = x @ wv, row-major [rows, 512] ---
                    psv_t = [ps_v.tile([P, DPC], F32, tag=f"v{jj}",
                                       name=f"psv{jj}")
                             for jj in range(MC // P)]
                    for k in range(NKT):
                        wvt = wv_pool.tile([P, DPC], BF16)
                        nc.sync.dma_start(wvt[:], wv_d[k * P:(k + 1) * P, :])
                        for jj in range(MC // P):
                            nc.tensor.matmul(
                                psv_t[jj][:],
                                xblk[:, k * MC + jj * P: k * MC + (jj + 1) * P],
                                wvt[:],
                                start=(k == 0), stop=(k == NKT - 1),
                            )
                    for jj in range(MC // P):
                        vout = vo_pool.tile([P, DPC], BF16)
                        nc.vector.tensor_copy(vout[:], psv_t[jj][:])
                        r0 = ms + jj * P
                        nc.sync.dma_start(v_d[r0:r0 + P, :], vout[:])

                    # --- qT / kT with fused RoPE ---
                    cq = trig_pool.tile([P, MC], F32, tag="cq")
                    sq = trig_pool.tile([P, MC], F32, tag="sq")
                    ck = trig_pool.tile([P, MC], F32, tag="ck")
                    sk = trig_pool.tile([P, MC], F32, tag="sk")
                    nc.sync.dma_start(cq[:], cosq_d[:, ms:ms + MC])
                    nc.sync.dma_start(sq[:], sinq_d[:, ms:ms + MC])
                    nc.sync.dma_start(ck[:], cosk_d[:, ms:ms + MC])
                    nc.sync.dma_start(sk[:], sink_d[:, ms:ms + MC])

                    for w_sb, cos_t, sin_t, dest in (
                        (wq_sb, cq, sq, qT_d),
                        (wk_sb, ck, sk, kT_d),
                    ):
                        for nt in range(DPC // P):
                            psq = ps_qk.tile([P, MC], F32)
                            for k in range(NKT):
                                nc.tensor.matmul(
                                    psq[:],
                                    w_sb[:, k * DPC + nt * P: k * DPC + (nt + 1) * P],
                                    xblk[:, k * MC:(k + 1) * MC],
                                    start=(k == 0), stop=(k == NKT - 1),
                                )
                            cp = rope_pool.tile([P, MC], F32, tag="cp")
                            nc.scalar.activation(cp[:], psq[:], CopyF)
                            rot = rope_pool.tile([P, MC], F32, tag="rot")
                            nc.sync.dma_start(rot[0:64, :], cp[64:128, :])
                            nc.sync.dma_start(rot[64:128, :], cp[0:64, :])
                            tmp = rope_pool.tile([P, MC], F32, tag="tmp")
                            nc.vector.tensor_mul(tmp[:], psq[:], cos_t[:])
                            nc.vector.tensor_mul(rot[:], rot[:], sin_t[:])
                            ob = qko_pool.tile([P, MC], BF16)
                            nc.vector.tensor_add(ob[:], tmp[:], rot[:])
                            nc.sync.dma_start(
                                dest[nt * P:(nt + 1) * P, ms:ms + MC], ob[:])

            # -------- Phase 2+3 fused: attention + o-projection ----------
            # h is the INNER loop so all four heads' normalized outputs for
            # one query chunk sit in SBUF; the partial o-projection for those
            # rows follows immediately (no oT DRAM round trip), giving the PE
            # independent work to fill the softmax dependency bubbles.
            with (
                tc.tile_pool(name="wo3", bufs=1) as wo_pool,
                tc.tile_pool(name="mask2", bufs=1) as mask_pool,
                tc.tile_pool(name="ost", bufs=2) as ost_pool,
                tc.tile_pool(name="qkv2", bufs=2) as qkv_pool,
                tc.tile_pool(name="expb", bufs=6) as exp_pool,
                tc.tile_pool(name="norm", bufs=3) as norm_pool,
                tc.tile_pool(name="ev", bufs=4) as ev_pool,
                tc.tile_pool(name="pss", bufs=3, space="PSUM") as ps_s,
                tc.tile_pool(name="pso", bufs=2, space="PSUM") as ps_o,
                tc.tile_pool(name="psc", bufs=1, space="PSUM") as ps_c,
                tc.tile_pool(name="psp", bufs=2, space="PSUM") as ps_p,
            ):
                mask_sb = mask_pool.tile([P, 4 * MC], BF16, tag="mask")
                nc.sync.dma_start(mask_sb[:], mask_d[:])
                wo_sb = wo_pool.tile([P, HPC * HS], BF16, tag="wo")
                wo_loaded = False
                for b in range(BS):
                    c0 = b * SL
                    qt, kt, vt = [], [], []
                    for h in range(HPC):
                        q_h = qkv_pool.tile([P, SL], BF16, tag=f"q{h}")
                        k_h = qkv_pool.tile([P, SL], BF16, tag=f"k{h}")
                        nc.sync.dma_start(
                            q_h[:], qT_d[h * P:(h + 1) * P, c0:c0 + SL])
                        nc.sync.dma_start(
                            k_h[:], kT_d[h * P:(h + 1) * P, c0:c0 + SL])
                        v_h = qkv_pool.tile([P, NJT * HD], BF16, tag=f"v{h}")
                        nc.sync.dma_start(
                            v_h[:].rearrange("p (j d) -> p j d", j=NJT),
                            v_d[c0:c0 + SL, h * HD:(h + 1) * HD]
                                .rearrange("(j p) d -> p j d", p=P),
                        )
                        qt.append(q_h); kt.append(k_h); vt.append(v_h)
                    if not wo_loaded:
                        # after the first head's q/k/v so the attention
                        # pipeline starts before this 4MB load
                        wo_loaded = True
                        nc.sync.dma_start(
                            wo_sb[:].rearrange("p (a c) -> p a c", a=HPC),
                            wo_d.rearrange("(a p) c -> p a c", p=P),
                        )
                    for ic in range(NIC):
                        njt = 4 * (ic + 1)
                        ost_ic = []
                        for h in range(HPC):
                            ps_out = ps_o.tile([P, MC], F32)
                            ps_sum = ps_c.tile([P, MC], F32)
                            for jt in range(njt):
                                ps_sc = ps_s.tile([P, MC], F32)
                                nc.tensor.matmul(
                                    ps_sc[:],
                                    kt[h][:, jt * P:(jt + 1) * P],
                                    qt[h][:, ic * MC:(ic + 1) * MC],
                                    start=True, stop=True,
                                )
                                et = exp_pool.tile([P, MC], BF16)
                                nc.scalar.activation(et[:], ps_sc[:], ExpF)
                                if jt >= 4 * ic:
                                    t = jt - 4 * ic
                                    nc.vector.tensor_mul(
                                        et[:], et[:],
                                        mask_sb[:, t * MC:(t + 1) * MC])
                                nc.tensor.matmul(
                                    ps_out[:],
                                    vt[h][:, jt * HD:(jt + 1) * HD],
                                    et[:],
                                    start=(jt == 0), stop=(jt == njt - 1),
                                )
                                nc.tensor.matmul(
                                    ps_sum[:],
                                    ones_sb[:],
                                    et[:],
                                    start=(jt == 0), stop=(jt == njt - 1),
                                )
                            bcast = norm_pool.tile([P, MC], F32, tag="bcast")
                            nc.vector.reciprocal(bcast[:], ps_sum[:])
                            ost = ost_pool.tile([P, MC], BF16, tag=f"ost{h}",
                                                name=f"ost{h}")
                            nc.vector.tensor_mul(
                                ost[:], ps_out[:], bcast[:])
                            ost_ic.append(ost)
                        # partial o-projection for rows [c0+ic*MC, +MC)
                        for ct in range(HS // P):
                            psp = ps_p.tile([P, MC], F32)
                            for h in range(HPC):
                                nc.tensor.matmul(
                                    psp[:],
                                    wo_sb[:, h * HS + ct * P:
                                          h * HS + (ct + 1) * P],
                                    ost_ic[h][:],
                                    start=(h == 0), stop=(h == HPC - 1),
                                )
                            ev = ev_pool.tile([P, MC], F32)
                            nc.any.tensor_copy(ev[:], psp[:])
                            nc.sync.dma_start(
                                out_d[ct * P:(ct + 1) * P,
                                      c0 + ic * MC:c0 + (ic + 1) * MC],
                                ev[:])

    nc.compile()
    return nc


# Inputs that are identical on every core: shipped to the device ONCE
# (column-sharded) and replicated on-device by an all_gather program.
REPLICATED = ("xT", "cosq", "sinq", "cosk", "sink", "mask")


def _host_tables(hidden_states, wq, wk, wv, wo):
    """Single-copy replicated tables + per-core weight-slice stacks."""
    x = np.asarray(hidden_states, dtype=np.float32).reshape(ROWS, HS)
    xT = np.ascontiguousarray(x.T).astype(BFNP)

    inv_freq = 1.0 / (ROPE_THETA ** (np.arange(0, HD, 2, dtype=np.float32) / HD))
    pos = np.arange(SL, dtype=np.float32)
    freqs = pos[:, None] * inv_freq[None, :]
    emb = np.concatenate([freqs, freqs], axis=1)          # [SL, HD]
    cosT = np.cos(emb).astype(np.float32).T               # [HD, SL]
    sinT = np.sin(emb).astype(np.float32).T
    sign = np.ones((HD, 1), np.float32)
    sign[:HD // 2] = -1.0
    cosq = np.ascontiguousarray(np.tile(cosT, (1, BS)) * SCALE)
    sinq = np.ascontiguousarray(np.tile(sinT, (1, BS)) * sign * SCALE)
    cosk = np.ascontiguousarray(np.tile(cosT, (1, BS)))
    sink = np.ascontiguousarray(np.tile(sinT, (1, BS)) * sign)

    jj = np.arange(P)[:, None]
    ii = np.arange(MC)[None, :]
    mask = np.concatenate(
        [(t * P + jj <= ii) for t in range(4)], axis=1).astype(BFNP)

    wq = np.asarray(wq, np.float32)
    wk = np.asarray(wk, np.float32)
    wv = np.asarray(wv, np.float32)
    wo = np.asarray(wo, np.float32)

    # weight stacks: [N_CORES*rows, cols] so P("core") slices per core
    wq_s = np.concatenate(
        [wq[:, c * DPC:(c + 1) * DPC] for c in range(N_CORES)], 0).astype(BFNP)
    wk_s = np.concatenate(
        [wk[:, c * DPC:(c + 1) * DPC] for c in range(N_CORES)], 0).astype(BFNP)
    wv_s = np.concatenate(
        [wv[:, c * DPC:(c + 1) * DPC] for c in range(N_CORES)], 0).astype(BFNP)
    wo_s = np.ascontiguousarray(wo).astype(BFNP)   # [N_CORES*DPC, HS] already

    return (
        {"xT": xT, "cosq": cosq, "sinq": sinq, "cosk": cosk, "sink": sink,
         "mask": mask},
        {"wq": wq_s, "wk": wk_s, "wv": wv_s, "wo": wo_s},
    )


class Runner:
    """Compile the program once into a sharded PJRT executable; reuse across
    calls (no donation, so output buffers can stay device-resident)."""

    def __init__(self, nc):
        import jax
        import concourse.mybir as _mybir
        from concourse import bass2jax
        from jax.experimental.shard_map import shard_map
        from jax.sharding import Mesh, PartitionSpec, NamedSharding

        bass2jax.install_neuronx_cc_hook()
        self.jax = jax
        partition_name = (
            nc.partition_id_tensor.name if nc.partition_id_tensor else None)
        in_names, out_names, out_avals, zero_outs = [], [], [], []
        for alloc in nc.m.functions[0].allocations:
            if not isinstance(alloc, _mybir.MemoryLocationSet):
                continue
            name = alloc.memorylocations[0].name
            if alloc.kind == "ExternalInput":
                if name != partition_name:
                    in_names.append(name)
            elif alloc.kind == "ExternalOutput":
                shape = tuple(alloc.tensor_shape)
                dtype = _mybir.dt.np(alloc.dtype)
                out_names.append(name)
                out_avals.append(jax.core.ShapedArray(shape, dtype))
                zero_outs.append(np.zeros(shape, dtype))
        self.in_names, self.out_names = in_names, out_names
        self.out_avals = out_avals
        all_names = list(in_names)
        if partition_name is not None:
            all_names = all_names + [partition_name]

        def _body(*args):
            operands = list(args)
            if partition_name is not None:
                operands.append(bass2jax.partition_id_tensor())
            outs = bass2jax._bass_exec_p.bind(
                *operands,
                out_avals=tuple(out_avals),
                in_names=tuple(all_names),
                out_names=tuple(out_names),
                lowering_input_output_aliases=(),
                sim_require_finite=True,
                sim_require_nnan=True,
                nc=nc,
            )
            return tuple(outs)

        devices = jax.devices()[:N_CORES]
        mesh = Mesh(np.asarray(devices), ("core",))
        self.mesh = mesh
        self.sharding = NamedSharding(mesh, PartitionSpec("core"))
        self.fn = jax.jit(
            shard_map(
                _body, mesh=mesh,
                in_specs=(PartitionSpec("core"),) * len(in_names),
                out_specs=(PartitionSpec("core"),) * len(out_names),
                check_rep=False,
            ),
            keep_unused=True,
        )

        # pre: on-device replication of shared tables (ship 1/8 of the bytes,
        # all_gather to a per-core full copy stacked on axis 0)
        col_sharding = NamedSharding(mesh, PartitionSpec(None, "core"))
        self.col_sharding = col_sharding

        def _pre_body(*cols):
            return tuple(
                jax.lax.all_gather(c, "core", axis=1, tiled=True)
                for c in cols)

        n_rep = len(REPLICATED)
        self.pre = jax.jit(
            shard_map(
                _pre_body, mesh=mesh,
                in_specs=(PartitionSpec(None, "core"),) * n_rep,
                out_specs=(PartitionSpec("core", None),) * n_rep,
                check_rep=False,
            ))

        # post: on-device 8-way reduction of the partial o-projections plus
        # transpose into the final [ROWS, HS] layout
        import jax.numpy as jnp

        def _post_body(p):
            r = jax.lax.psum_scatter(p, "core", scatter_dimension=0,
                                     tiled=True)     # [DPC, ROWS] f32
            return jnp.transpose(r)                  # [ROWS, DPC]

        self.post = jax.jit(
            shard_map(
                _post_body, mesh=mesh,
                in_specs=(PartitionSpec("core", None),),
                out_specs=PartitionSpec(None, "core"),
                check_rep=False,
            ))

    def prepare_device_args(self, tables, weights):
        """Ship inputs to the cores: shared tables column-sharded then
        all-gathered on device; weight stacks row-sharded directly."""
        jax = self.jax
        rep_dev = self.pre(*[
            jax.device_put(tables[n], self.col_sharding) for n in REPLICATED])
        by_name = dict(zip(REPLICATED, rep_dev))
        for n, w in weights.items():
            by_name[n] = jax.device_put(w, self.sharding)
        return [by_name[n] for n in self.in_names]

    def finalize(self, out_arrs):
        """Reduce the 8 partial o-projections on device; fetch [ROWS, HS]."""
        return np.asarray(self.post(out_arrs[0]))


_RUNNER = None


def get_runner():
    global _RUNNER
    if _RUNNER is None:
        _RUNNER = Runner(build_program())
    return _RUNNER


def kernel(hidden_states, wq, wk, wv, wo):
    runner = get_runner()
    tables, weights = _host_tables(hidden_states, wq, wk, wv, wo)
    dev_args = runner.prepare_device_args(tables, weights)
    out_arrs = runner.fn(*dev_args)
    return runner.finalize(out_arrs).reshape(BS, SL, HS)



# revision 23
# speedup vs baseline: 1.0988x; 1.0988x over previous
"""Multi-head causal self-attention (32 heads, RoPE) on 8 Trainium2 cores.

Tensor-parallel over heads: core c owns heads 4c..4c+3 (512 of 4096 qkv dims).
Each core computes q/k/v projections for its heads, RoPE, causal softmax
attention fused with its partial o-projection (per query chunk, straight from
SBUF); the 8 partials are reduce-scattered on device.

Data movement: shared inputs (xT, RoPE tables, mask) are shipped to the
device once, column-sharded, and replicated on-device via all_gather; the
partial outputs are combined by an on-device psum_scatter + transpose so only
the final [rows, hs] f32 result crosses the (slow) axon tunnel.

Layouts (per core):
  xT    [4096 hs, 4096 rows]  bf16   rows = b*2048 + t
  qT/kT [512 d, 4096 rows]    bf16   (transposed: head dim on partitions)
  v     [4096 rows, 512 d]    bf16   (row-major)
  out   [4096 cols, 4096 rows] f32   partial of (attn_out @ wo)^T

Softmax runs on transposed scores sT[j,i] (keys on partitions): no-max-sub
exp (scores ~N(0,1)), column sums via ones-matmul on the PE, late
normalization with a partition-broadcast reciprocal.
"""
import sys

for _p in ("/opt/trn_rl_repo", "/root/.axon_site/_ro/trn_rl_repo"):
    if _p not in sys.path:
        sys.path.append(_p)

import numpy as np
import ml_dtypes

import concourse.bacc as bacc
import concourse.mybir as mybir
import concourse.tile as tile

BF16 = mybir.dt.bfloat16
F32 = mybir.dt.float32
BFNP = ml_dtypes.bfloat16

N_CORES = 8
BS, SL, HS = 2, 2048, 4096
NH, HD = 32, 128
HPC = NH // N_CORES          # heads per core = 4
DPC = HPC * HD               # qkv dims per core = 512
ROWS = BS * SL               # 4096
P = 128
MC = 512                     # m-chunk (rows) width
NMC = ROWS // MC             # 8 m-chunks
NKT = HS // P                # 32 contraction tiles
NIC = SL // MC               # 4 query chunks per sequence
NJT = SL // P                # 16 key tiles per sequence
SCALE = float(HD) ** -0.5
ROPE_THETA = 10000.0

ExpF = mybir.ActivationFunctionType.Exp
CopyF = mybir.ActivationFunctionType.Copy


def build_program():
    nc = bacc.Bacc("TRN2", target_bir_lowering=False, debug=False,
                   num_devices=N_CORES)

    xT_d = nc.dram_tensor("xT", [HS, ROWS], BF16, kind="ExternalInput").ap()
    wq_d = nc.dram_tensor("wq", [HS, DPC], BF16, kind="ExternalInput").ap()
    wk_d = nc.dram_tensor("wk", [HS, DPC], BF16, kind="ExternalInput").ap()
    wv_d = nc.dram_tensor("wv", [HS, DPC], BF16, kind="ExternalInput").ap()
    wo_d = nc.dram_tensor("wo", [DPC, HS], BF16, kind="ExternalInput").ap()
    cosq_d = nc.dram_tensor("cosq", [P, ROWS], F32, kind="ExternalInput").ap()
    sinq_d = nc.dram_tensor("sinq", [P, ROWS], F32, kind="ExternalInput").ap()
    cosk_d = nc.dram_tensor("cosk", [P, ROWS], F32, kind="ExternalInput").ap()
    sink_d = nc.dram_tensor("sink", [P, ROWS], F32, kind="ExternalInput").ap()
    mask_d = nc.dram_tensor("mask", [P, 4 * MC], BF16, kind="ExternalInput").ap()
    out_d = nc.dram_tensor("out", [HS, ROWS], F32, kind="ExternalOutput").ap()

    qT_d = nc.dram_tensor("qT_i", [DPC, ROWS], BF16).ap()
    kT_d = nc.dram_tensor("kT_i", [DPC, ROWS], BF16).ap()
    v_d = nc.dram_tensor("v_i", [ROWS, DPC], BF16).ap()

    with tile.TileContext(nc) as tc:
        with tc.tile_pool(name="const", bufs=1) as const_pool:
            ones_sb = const_pool.tile([P, P], BF16, tag="ones")
            nc.vector.memset(ones_sb[:], 1.0)

            # ---------------- Phase 1: q/k/v projections + RoPE ----------
            with (
                tc.tile_pool(name="wqk", bufs=1) as wqk_pool,
                tc.tile_pool(name="xb", bufs=2) as x_pool,
                tc.tile_pool(name="wvt", bufs=3) as wv_pool,
                tc.tile_pool(name="trig", bufs=2) as trig_pool,
                tc.tile_pool(name="rope", bufs=3) as rope_pool,
                tc.tile_pool(name="qko", bufs=4) as qko_pool,
                tc.tile_pool(name="vo", bufs=3) as vo_pool,
                tc.tile_pool(name="psv", bufs=1, space="PSUM") as ps_v,
                tc.tile_pool(name="psqk", bufs=2, space="PSUM") as ps_qk,
            ):
                wq_sb = wqk_pool.tile([P, NKT * DPC], BF16, tag="wq")
                wk_sb = wqk_pool.tile([P, NKT * DPC], BF16, tag="wk")
                xblk0 = x_pool.tile([P, NKT * MC], BF16, tag="xblk")
                # chunk-0 x gets the DMA engines to itself (wq/wk gated on
                # its completion) so the v-matmul chain starts ~13us in;
                # wq/wk then stream under the chunk-0 compute.
                xb0_dma = nc.sync.dma_start(
                    xblk0[:].rearrange("p (k m) -> p k m", k=NKT),
                    xT_d[:, 0:MC].rearrange("(k p) m -> p k m", p=P),
                )
                wq_dma = nc.sync.dma_start(
                    wq_sb[:].rearrange("p (k n) -> p k n", k=NKT),
                    wq_d.rearrange("(k p) n -> p k n", p=P),
                )
                wk_dma = nc.sync.dma_start(
                    wk_sb[:].rearrange("p (k n) -> p k n", k=NKT),
                    wk_d.rearrange("(k p) n -> p k n", p=P),
                )
                from concourse.tile import add_dep_helper
                for dep in (wq_dma, wk_dma):
                    add_dep_helper(
                        getattr(dep, "ins", dep), getattr(xb0_dma, "ins", xb0_dma),
                        reason="let chunk-0 x win the DMA bandwidth race")

                for mc in range(NMC):
                    ms = mc * MC
                    if mc == 0:
                        xblk = xblk0
                    else:
                        xblk = x_pool.tile([P, NKT * MC], BF16, tag="xblk")
                        nc.sync.dma_start(
                            xblk[:].rearrange("p (k m) -> p k m", k=NKT),
                            xT_d[:, ms:ms + MC].rearrange("(k p) m -> p k m", p=P),
                        )
                    # --- v = x @ wv, row-major [rows, 512] ---
                    psv_t = [ps_v.tile([P, DPC], F32, tag=f"v{jj}",
                                       name=f"psv{jj}")
                             for jj in range(MC // P)]
                    for k in range(NKT):
                        wvt = wv_pool.tile([P, DPC], BF16)
                        nc.sync.dma_start(wvt[:], wv_d[k * P:(k + 1) * P, :])
                        for jj in range(MC // P):
                            nc.tensor.matmul(
                                psv_t[jj][:],
                                xblk[:, k * MC + jj * P: k * MC + (jj + 1) * P],
                                wvt[:],
                                start=(k == 0), stop=(k == NKT - 1),
                            )
                    for jj in range(MC // P):
                        vout = vo_pool.tile([P, DPC], BF16)
                        nc.vector.tensor_copy(vout[:], psv_t[jj][:])
                        r0 = ms + jj * P
                        nc.sync.dma_start(v_d[r0:r0 + P, :], vout[:])

                    # --- qT / kT with fused RoPE ---
                    cq = trig_pool.tile([P, MC], F32, tag="cq")
                    sq = trig_pool.tile([P, MC], F32, tag="sq")
                    ck = trig_pool.tile([P, MC], F32, tag="ck")
                    sk = trig_pool.tile([P, MC], F32, tag="sk")
                    nc.sync.dma_start(cq[:], cosq_d[:, ms:ms + MC])
                    nc.sync.dma_start(sq[:], sinq_d[:, ms:ms + MC])
                    nc.sync.dma_start(ck[:], cosk_d[:, ms:ms + MC])
                    nc.sync.dma_start(sk[:], sink_d[:, ms:ms + MC])

                    for w_sb, cos_t, sin_t, dest in (
                        (wq_sb, cq, sq, qT_d),
                        (wk_sb, ck, sk, kT_d),
                    ):
                        for nt in range(DPC // P):
                            psq = ps_qk.tile([P, MC], F32)
                            for k in range(NKT):
                                nc.tensor.matmul(
                                    psq[:],
                                    w_sb[:, k * DPC + nt * P: k * DPC + (nt + 1) * P],
                                    xblk[:, k * MC:(k + 1) * MC],
                                    start=(k == 0), stop=(k == NKT - 1),
                                )
                            cp = rope_pool.tile([P, MC], F32, tag="cp")
                            nc.scalar.activation(cp[:], psq[:], CopyF)
                            rot = rope_pool.tile([P, MC], F32, tag="rot")
                            nc.sync.dma_start(rot[0:64, :], cp[64:128, :])
                            nc.sync.dma_start(rot[64:128, :], cp[0:64, :])
                            tmp = rope_pool.tile([P, MC], F32, tag="tmp")
                            nc.vector.tensor_mul(tmp[:], psq[:], cos_t[:])
                            nc.vector.tensor_mul(rot[:], rot[:], sin_t[:])
                            ob = qko_pool.tile([P, MC], BF16)
                            nc.vector.tensor_add(ob[:], tmp[:], rot[:])
                            nc.sync.dma_start(
                                dest[nt * P:(nt + 1) * P, ms:ms + MC], ob[:])

            # -------- Phase 2+3 fused: attention + o-projection ----------
            # h is the INNER loop so all four heads' normalized outputs for
            # one query chunk sit in SBUF; the partial o-projection for those
            # rows follows immediately (no oT DRAM round trip), giving the PE
            # independent work to fill the softmax dependency bubbles.
            with (
                tc.tile_pool(name="wo3", bufs=1) as wo_pool,
                tc.tile_pool(name="mask2", bufs=1) as mask_pool,
                tc.tile_pool(name="ost", bufs=2) as ost_pool,
                tc.tile_pool(name="qkv2", bufs=2) as qkv_pool,
                tc.tile_pool(name="expb", bufs=6) as exp_pool,
                tc.tile_pool(name="norm", bufs=3) as norm_pool,
                tc.tile_pool(name="ev", bufs=4) as ev_pool,
                tc.tile_pool(name="pss", bufs=3, space="PSUM") as ps_s,
                tc.tile_pool(name="pso", bufs=2, space="PSUM") as ps_o,
                tc.tile_pool(name="psc", bufs=1, space="PSUM") as ps_c,
                tc.tile_pool(name="psp", bufs=2, space="PSUM") as ps_p,
            ):
                mask_sb = mask_pool.tile([P, 4 * MC], BF16, tag="mask")
                nc.sync.dma_start(mask_sb[:], mask_d[:])
                wo_sb = wo_pool.tile([P, HPC * HS], BF16, tag="wo")
                wo_loaded = False

                def oproj(po, pcol, ct):
                    """One 128-col slice of the partial o-projection for the
                    query chunk whose normalized heads are in `po`."""
                    psp = ps_p.tile([P, MC], F32)
                    for hh in range(HPC):
                        nc.tensor.matmul(
                            psp[:],
                            wo_sb[:, hh * HS + ct * P: hh * HS + (ct + 1) * P],
                            po[hh][:],
                            start=(hh == 0), stop=(hh == HPC - 1),
                        )
                    ev = ev_pool.tile([P, MC], F32)
                    nc.any.tensor_copy(ev[:], psp[:])
                    nc.sync.dma_start(
                        out_d[ct * P:(ct + 1) * P, pcol:pcol + MC], ev[:])

                # o-projection of chunk N is emitted interleaved with the
                # attention of chunk N+1 (8 col-slices after each head) so
                # its matmuls fill the exp-wait bubbles on the PE
                pending = None
                for b in range(BS):
                    c0 = b * SL
                    qt, kt, vt = [], [], []
                    for h in range(HPC):
                        q_h = qkv_pool.tile([P, SL], BF16, tag=f"q{h}")
                        k_h = qkv_pool.tile([P, SL], BF16, tag=f"k{h}")
                        nc.sync.dma_start(
                            q_h[:], qT_d[h * P:(h + 1) * P, c0:c0 + SL])
                        nc.sync.dma_start(
                            k_h[:], kT_d[h * P:(h + 1) * P, c0:c0 + SL])
                        v_h = qkv_pool.tile([P, NJT * HD], BF16, tag=f"v{h}")
                        nc.sync.dma_start(
                            v_h[:].rearrange("p (j d) -> p j d", j=NJT),
                            v_d[c0:c0 + SL, h * HD:(h + 1) * HD]
                                .rearrange("(j p) d -> p j d", p=P),
                        )
                        qt.append(q_h); kt.append(k_h); vt.append(v_h)
                    if not wo_loaded:
                        # after the first head's q/k/v so the attention
                        # pipeline starts before this 4MB load
                        wo_loaded = True
                        nc.sync.dma_start(
                            wo_sb[:].rearrange("p (a c) -> p a c", a=HPC),
                            wo_d.rearrange("(a p) c -> p a c", p=P),
                        )
                    for ic in range(NIC):
                        njt = 4 * (ic + 1)
                        ost_ic = []
                        for h in range(HPC):
                            ps_out = ps_o.tile([P, MC], F32)
                            # exp tiles are pre-summed on the DVE in f32 so
                            # the partition-reduce needs ONE ones-matmul per
                            # (b, ic, h) instead of one per key tile; two
                            # interleaved chains halve the serial latency
                            es32 = norm_pool.tile([P, MC], F32, tag="es32")
                            es32b = norm_pool.tile([P, MC], F32, tag="es32b")
                            for jt in range(njt):
                                ps_sc = ps_s.tile([P, MC], F32)
                                nc.tensor.matmul(
                                    ps_sc[:],
                                    kt[h][:, jt * P:(jt + 1) * P],
                                    qt[h][:, ic * MC:(ic + 1) * MC],
                                    start=True, stop=True,
                                )
                                et = exp_pool.tile([P, MC], BF16)
                                nc.scalar.activation(et[:], ps_sc[:], ExpF)
                                if jt >= 4 * ic:
                                    t = jt - 4 * ic
                                    nc.vector.tensor_mul(
                                        et[:], et[:],
                                        mask_sb[:, t * MC:(t + 1) * MC])
                                nc.tensor.matmul(
                                    ps_out[:],
                                    vt[h][:, jt * HD:(jt + 1) * HD],
                                    et[:],
                                    start=(jt == 0), stop=(jt == njt - 1),
                                )
                                # two chains on two engines: DVE takes even
                                # key tiles, the otherwise-idle GpSimd odd
                                eng = nc.vector if jt % 2 == 0 else nc.gpsimd
                                acc = es32 if jt % 2 == 0 else es32b
                                if jt < 2:
                                    eng.tensor_copy(acc[:], et[:])
                                else:
                                    eng.tensor_add(acc[:], acc[:], et[:])
                            nc.vector.tensor_add(es32[:], es32[:], es32b[:])
                            es16 = exp_pool.tile([P, MC], BF16, tag="es16")
                            nc.scalar.activation(es16[:], es32[:], CopyF)
                            ps_sum = ps_c.tile([P, MC], F32)
                            nc.tensor.matmul(
                                ps_sum[:], ones_sb[:], es16[:],
                                start=True, stop=True,
                            )
                            bcast = norm_pool.tile([P, MC], F32, tag="bcast")
                            nc.vector.reciprocal(bcast[:], ps_sum[:])
                            ost = ost_pool.tile([P, MC], BF16, tag=f"ost{h}",
                                                name=f"ost{h}")
                            nc.vector.tensor_mul(
                                ost[:], ps_out[:], bcast[:])
                            ost_ic.append(ost)
                            if pending is not None:
                                po, pcol = pending
                                for ct in range(h * 8, (h + 1) * 8):
                                    oproj(po, pcol, ct)
                        pending = (ost_ic, c0 + ic * MC)
                # flush the last chunk's o-projection
                po, pcol = pending
                for ct in range(HS // P):
                    oproj(po, pcol, ct)

    nc.compile()
    return nc


# Inputs that are identical on every core: shipped to the device ONCE
# (column-sharded) and replicated on-device by an all_gather program.
REPLICATED = ("xT", "cosq", "sinq", "cosk", "sink", "mask")


def _host_tables(hidden_states, wq, wk, wv, wo):
    """Single-copy replicated tables + per-core weight-slice stacks."""
    x = np.asarray(hidden_states, dtype=np.float32).reshape(ROWS, HS)
    xT = np.ascontiguousarray(x.T).astype(BFNP)

    inv_freq = 1.0 / (ROPE_THETA ** (np.arange(0, HD, 2, dtype=np.float32) / HD))
    pos = np.arange(SL, dtype=np.float32)
    freqs = pos[:, None] * inv_freq[None, :]
    emb = np.concatenate([freqs, freqs], axis=1)          # [SL, HD]
    cosT = np.cos(emb).astype(np.float32).T               # [HD, SL]
    sinT = np.sin(emb).astype(np.float32).T
    sign = np.ones((HD, 1), np.float32)
    sign[:HD // 2] = -1.0
    cosq = np.ascontiguousarray(np.tile(cosT, (1, BS)) * SCALE)
    sinq = np.ascontiguousarray(np.tile(sinT, (1, BS)) * sign * SCALE)
    cosk = np.ascontiguousarray(np.tile(cosT, (1, BS)))
    sink = np.ascontiguousarray(np.tile(sinT, (1, BS)) * sign)

    jj = np.arange(P)[:, None]
    ii = np.arange(MC)[None, :]
    mask = np.concatenate(
        [(t * P + jj <= ii) for t in range(4)], axis=1).astype(BFNP)

    wq = np.asarray(wq, np.float32)
    wk = np.asarray(wk, np.float32)
    wv = np.asarray(wv, np.float32)
    wo = np.asarray(wo, np.float32)

    # weight stacks: [N_CORES*rows, cols] so P("core") slices per core
    wq_s = np.concatenate(
        [wq[:, c * DPC:(c + 1) * DPC] for c in range(N_CORES)], 0).astype(BFNP)
    wk_s = np.concatenate(
        [wk[:, c * DPC:(c + 1) * DPC] for c in range(N_CORES)], 0).astype(BFNP)
    wv_s = np.concatenate(
        [wv[:, c * DPC:(c + 1) * DPC] for c in range(N_CORES)], 0).astype(BFNP)
    wo_s = np.ascontiguousarray(wo).astype(BFNP)   # [N_CORES*DPC, HS] already

    return (
        {"xT": xT, "cosq": cosq, "sinq": sinq, "cosk": cosk, "sink": sink,
         "mask": mask},
        {"wq": wq_s, "wk": wk_s, "wv": wv_s, "wo": wo_s},
    )


class Runner:
    """Compile the program once into a sharded PJRT executable; reuse across
    calls (no donation, so output buffers can stay device-resident)."""

    def __init__(self, nc):
        import jax
        import concourse.mybir as _mybir
        from concourse import bass2jax
        from jax.experimental.shard_map import shard_map
        from jax.sharding import Mesh, PartitionSpec, NamedSharding

        bass2jax.install_neuronx_cc_hook()
        self.jax = jax
        partition_name = (
            nc.partition_id_tensor.name if nc.partition_id_tensor else None)
        in_names, out_names, out_avals, zero_outs = [], [], [], []
        for alloc in nc.m.functions[0].allocations:
            if not isinstance(alloc, _mybir.MemoryLocationSet):
                continue
            name = alloc.memorylocations[0].name
            if alloc.kind == "ExternalInput":
                if name != partition_name:
                    in_names.append(name)
            elif alloc.kind == "ExternalOutput":
                shape = tuple(alloc.tensor_shape)
                dtype = _mybir.dt.np(alloc.dtype)
                out_names.append(name)
                out_avals.append(jax.core.ShapedArray(shape, dtype))
                zero_outs.append(np.zeros(shape, dtype))
        self.in_names, self.out_names = in_names, out_names
        self.out_avals = out_avals
        all_names = list(in_names)
        if partition_name is not None:
            all_names = all_names + [partition_name]

        def _body(*args):
            operands = list(args)
            if partition_name is not None:
                operands.append(bass2jax.partition_id_tensor())
            outs = bass2jax._bass_exec_p.bind(
                *operands,
                out_avals=tuple(out_avals),
                in_names=tuple(all_names),
                out_names=tuple(out_names),
                lowering_input_output_aliases=(),
                sim_require_finite=True,
                sim_require_nnan=True,
                nc=nc,
            )
            return tuple(outs)

        devices = jax.devices()[:N_CORES]
        mesh = Mesh(np.asarray(devices), ("core",))
        self.mesh = mesh
        self.sharding = NamedSharding(mesh, PartitionSpec("core"))
        self.fn = jax.jit(
            shard_map(
                _body, mesh=mesh,
                in_specs=(PartitionSpec("core"),) * len(in_names),
                out_specs=(PartitionSpec("core"),) * len(out_names),
                check_rep=False,
            ),
            keep_unused=True,
        )

        # pre: on-device replication of shared tables (ship 1/8 of the bytes,
        # all_gather to a per-core full copy stacked on axis 0)
        col_sharding = NamedSharding(mesh, PartitionSpec(None, "core"))
        self.col_sharding = col_sharding

        def _pre_body(*cols):
            return tuple(
                jax.lax.all_gather(c, "core", axis=1, tiled=True)
                for c in cols)

        n_rep = len(REPLICATED)
        self.pre = jax.jit(
            shard_map(
                _pre_body, mesh=mesh,
                in_specs=(PartitionSpec(None, "core"),) * n_rep,
                out_specs=(PartitionSpec("core", None),) * n_rep,
                check_rep=False,
            ))

        # post: on-device 8-way reduction of the partial o-projections plus
        # transpose into the final [ROWS, HS] layout
        import jax.numpy as jnp

        def _post_body(p):
            r = jax.lax.psum_scatter(p, "core", scatter_dimension=0,
                                     tiled=True)     # [DPC, ROWS] f32
            return jnp.transpose(r)                  # [ROWS, DPC]

        self.post = jax.jit(
            shard_map(
                _post_body, mesh=mesh,
                in_specs=(PartitionSpec("core", None),),
                out_specs=PartitionSpec(None, "core"),
                check_rep=False,
            ))

    def prepare_device_args(self, tables, weights):
        """Ship inputs to the cores: shared tables column-sharded then
        all-gathered on device; weight stacks row-sharded directly."""
        jax = self.jax
        rep_dev = self.pre(*[
            jax.device_put(tables[n], self.col_sharding) for n in REPLICATED])
        by_name = dict(zip(REPLICATED, rep_dev))
        for n, w in weights.items():
            by_name[n] = jax.device_put(w, self.sharding)
        return [by_name[n] for n in self.in_names]

    def finalize(self, out_arrs):
        """Reduce the 8 partial o-projections on device; fetch [ROWS, HS]."""
        return np.asarray(self.post(out_arrs[0]))


_RUNNER = None


def get_runner():
    global _RUNNER
    if _RUNNER is None:
        _RUNNER = Runner(build_program())
    return _RUNNER


def kernel(hidden_states, wq, wk, wv, wo):
    runner = get_runner()
    tables, weights = _host_tables(hidden_states, wq, wk, wv, wo)
    dev_args = runner.prepare_device_args(tables, weights)
    out_arrs = runner.fn(*dev_args)
    return runner.finalize(out_arrs).reshape(BS, SL, HS)



# revision 29
# speedup vs baseline: 1.1576x; 1.0535x over previous
"""Multi-head causal self-attention (32 heads, RoPE) on 8 Trainium2 cores.

Tensor-parallel over heads: core c owns heads 4c..4c+3 (512 of 4096 qkv dims).
Each core computes q/k/v projections for its heads, RoPE, causal softmax
attention fused with its partial o-projection (per query chunk, straight from
SBUF); the 8 partials are reduce-scattered on device.

Data movement: shared inputs (xT, RoPE tables, mask) are shipped to the
device once, column-sharded, and replicated on-device via all_gather; the
partial outputs are combined by an on-device psum_scatter + transpose so only
the final [rows, hs] f32 result crosses the (slow) axon tunnel.

Layouts (per core):
  xT    [4096 hs, 4096 rows]  bf16   rows = b*2048 + t
  qT/kT [512 d, 4096 rows]    bf16   (transposed: head dim on partitions)
  v     [4096 rows, 512 d]    bf16   (row-major)
  out   [4096 cols, 4096 rows] f32   partial of (attn_out @ wo)^T

Softmax runs on transposed scores sT[j,i] (keys on partitions): no-max-sub
exp (scores ~N(0,1)), column sums via ones-matmul on the PE, late
normalization with a partition-broadcast reciprocal.
"""
import sys

for _p in ("/opt/trn_rl_repo", "/root/.axon_site/_ro/trn_rl_repo"):
    if _p not in sys.path:
        sys.path.append(_p)

import numpy as np
import ml_dtypes

import concourse.bacc as bacc
import concourse.mybir as mybir
import concourse.tile as tile

BF16 = mybir.dt.bfloat16
F32 = mybir.dt.float32
BFNP = ml_dtypes.bfloat16

N_CORES = 8
BS, SL, HS = 2, 2048, 4096
NH, HD = 32, 128
HPC = NH // N_CORES          # heads per core = 4
DPC = HPC * HD               # qkv dims per core = 512
ROWS = BS * SL               # 4096
P = 128
MC = 512                     # m-chunk (rows) width
NMC = ROWS // MC             # 8 m-chunks
NKT = HS // P                # 32 contraction tiles
NIC = SL // MC               # 4 query chunks per sequence
NJT = SL // P                # 16 key tiles per sequence
SCALE = float(HD) ** -0.5
ROPE_THETA = 10000.0

ExpF = mybir.ActivationFunctionType.Exp
CopyF = mybir.ActivationFunctionType.Copy


def build_program():
    nc = bacc.Bacc("TRN2", target_bir_lowering=False, debug=False,
                   num_devices=N_CORES)

    xT_d = nc.dram_tensor("xT", [HS, ROWS], BF16, kind="ExternalInput").ap()
    wq_d = nc.dram_tensor("wq", [HS, DPC], BF16, kind="ExternalInput").ap()
    wk_d = nc.dram_tensor("wk", [HS, DPC], BF16, kind="ExternalInput").ap()
    wv_d = nc.dram_tensor("wv", [HS, DPC], BF16, kind="ExternalInput").ap()
    wo_d = nc.dram_tensor("wo", [DPC, HS], BF16, kind="ExternalInput").ap()
    cosq_d = nc.dram_tensor("cosq", [P, ROWS], F32, kind="ExternalInput").ap()
    sinq_d = nc.dram_tensor("sinq", [P, ROWS], F32, kind="ExternalInput").ap()
    cosk_d = nc.dram_tensor("cosk", [P, ROWS], F32, kind="ExternalInput").ap()
    sink_d = nc.dram_tensor("sink", [P, ROWS], F32, kind="ExternalInput").ap()
    mask_d = nc.dram_tensor("mask", [P, 4 * MC], BF16, kind="ExternalInput").ap()
    out_d = nc.dram_tensor("out", [HS, ROWS], F32, kind="ExternalOutput").ap()

    qT_d = nc.dram_tensor("qT_i", [DPC, ROWS], BF16).ap()
    kT_d = nc.dram_tensor("kT_i", [DPC, ROWS], BF16).ap()
    v_d = nc.dram_tensor("v_i", [ROWS, DPC], BF16).ap()

    with tile.TileContext(nc) as tc:
        with tc.tile_pool(name="const", bufs=1) as const_pool:
            ones_sb = const_pool.tile([P, P], BF16, tag="ones")
            nc.vector.memset(ones_sb[:], 1.0)

            # ---------------- Phase 1: q/k/v projections + RoPE ----------
            with (
                tc.tile_pool(name="wqk", bufs=1) as wqk_pool,
                tc.tile_pool(name="xb", bufs=2) as x_pool,
                tc.tile_pool(name="trig", bufs=2) as trig_pool,
                tc.tile_pool(name="rope", bufs=3) as rope_pool,
                tc.tile_pool(name="qko", bufs=4) as qko_pool,
                tc.tile_pool(name="vo", bufs=3) as vo_pool,
                tc.tile_pool(name="psv", bufs=1, space="PSUM") as ps_v,
                tc.tile_pool(name="psqk", bufs=2, space="PSUM") as ps_qk,
            ):
                wq_sb = wqk_pool.tile([P, NKT * DPC], BF16, tag="wq")
                wk_sb = wqk_pool.tile([P, NKT * DPC], BF16, tag="wk")
                wv_sb = wqk_pool.tile([P, NKT * DPC], BF16, tag="wv")
                xblk0 = x_pool.tile([P, NKT * MC], BF16, tag="xblk")
                # chunk-0 x and the first wv slice get the DMA engines first
                # (wq/wk gated on chunk-0 x) so the v-matmul chain starts
                # ~14us in; the rest streams under the chunk-0 compute.
                # wv stays SBUF-resident for the whole phase (saves 28MB of
                # per-chunk re-reads and their fixed-cost pacing).
                xb0_dma = nc.sync.dma_start(
                    xblk0[:].rearrange("p (k m) -> p k m", k=NKT),
                    xT_d[:, 0:MC].rearrange("(k p) m -> p k m", p=P),
                )
                NKW = 4    # first wv slice: k-tiles 0..3
                nc.sync.dma_start(
                    wv_sb[:, :NKW * DPC].rearrange("p (k n) -> p k n", k=NKW),
                    wv_d[:NKW * P, :].rearrange("(k p) n -> p k n", p=P),
                )
                nc.sync.dma_start(
                    wv_sb[:, NKW * DPC:].rearrange("p (k n) -> p k n",
                                                   k=NKT - NKW),
                    wv_d[NKW * P:, :].rearrange("(k p) n -> p k n", p=P),
                )
                wq_dma = nc.sync.dma_start(
                    wq_sb[:].rearrange("p (k n) -> p k n", k=NKT),
                    wq_d.rearrange("(k p) n -> p k n", p=P),
                )
                wk_dma = nc.sync.dma_start(
                    wk_sb[:].rearrange("p (k n) -> p k n", k=NKT),
                    wk_d.rearrange("(k p) n -> p k n", p=P),
                )
                from concourse.tile import add_dep_helper
                for dep in (wq_dma, wk_dma):
                    add_dep_helper(
                        getattr(dep, "ins", dep), getattr(xb0_dma, "ins", xb0_dma),
                        reason="let chunk-0 x win the DMA bandwidth race")

                for mc in range(NMC):
                    ms = mc * MC
                    if mc == 0:
                        xblk = xblk0
                    else:
                        xblk = x_pool.tile([P, NKT * MC], BF16, tag="xblk")
                        nc.sync.dma_start(
                            xblk[:].rearrange("p (k m) -> p k m", k=NKT),
                            xT_d[:, ms:ms + MC].rearrange("(k p) m -> p k m", p=P),
                        )
                    # --- v = x @ wv, row-major [rows, 512] ---
                    psv_t = [ps_v.tile([P, DPC], F32, tag=f"v{jj}",
                                       name=f"psv{jj}")
                             for jj in range(MC // P)]
                    for k in range(NKT):
                        for jj in range(MC // P):
                            nc.tensor.matmul(
                                psv_t[jj][:],
                                xblk[:, k * MC + jj * P: k * MC + (jj + 1) * P],
                                wv_sb[:, k * DPC:(k + 1) * DPC],
                                start=(k == 0), stop=(k == NKT - 1),
                            )
                    for jj in range(MC // P):
                        vout = vo_pool.tile([P, DPC], BF16)
                        nc.vector.tensor_copy(vout[:], psv_t[jj][:])
                        r0 = ms + jj * P
                        nc.sync.dma_start(v_d[r0:r0 + P, :], vout[:])

                    # --- qT / kT with fused RoPE ---
                    cq = trig_pool.tile([P, MC], F32, tag="cq")
                    sq = trig_pool.tile([P, MC], F32, tag="sq")
                    ck = trig_pool.tile([P, MC], F32, tag="ck")
                    sk = trig_pool.tile([P, MC], F32, tag="sk")
                    nc.sync.dma_start(cq[:], cosq_d[:, ms:ms + MC])
                    nc.sync.dma_start(sq[:], sinq_d[:, ms:ms + MC])
                    nc.sync.dma_start(ck[:], cosk_d[:, ms:ms + MC])
                    nc.sync.dma_start(sk[:], sink_d[:, ms:ms + MC])

                    for w_sb, cos_t, sin_t, dest in (
                        (wq_sb, cq, sq, qT_d),
                        (wk_sb, ck, sk, kT_d),
                    ):
                        for nt in range(DPC // P):
                            psq = ps_qk.tile([P, MC], F32)
                            for k in range(NKT):
                                nc.tensor.matmul(
                                    psq[:],
                                    w_sb[:, k * DPC + nt * P: k * DPC + (nt + 1) * P],
                                    xblk[:, k * MC:(k + 1) * MC],
                                    start=(k == 0), stop=(k == NKT - 1),
                                )
                            cp = rope_pool.tile([P, MC], F32, tag="cp")
                            nc.scalar.activation(cp[:], psq[:], CopyF)
                            rot = rope_pool.tile([P, MC], F32, tag="rot")
                            nc.sync.dma_start(rot[0:64, :], cp[64:128, :])
                            nc.sync.dma_start(rot[64:128, :], cp[0:64, :])
                            tmp = rope_pool.tile([P, MC], F32, tag="tmp")
                            nc.vector.tensor_mul(tmp[:], psq[:], cos_t[:])
                            nc.vector.tensor_mul(rot[:], rot[:], sin_t[:])
                            ob = qko_pool.tile([P, MC], BF16)
                            nc.vector.tensor_add(ob[:], tmp[:], rot[:])
                            nc.sync.dma_start(
                                dest[nt * P:(nt + 1) * P, ms:ms + MC], ob[:])

            # -------- Phase 2+3 fused: attention + o-projection ----------
            # h is the INNER loop so all four heads' normalized outputs for
            # one query chunk sit in SBUF; the partial o-projection for those
            # rows follows immediately (no oT DRAM round trip), giving the PE
            # independent work to fill the softmax dependency bubbles.
            with (
                tc.tile_pool(name="wo3", bufs=1) as wo_pool,
                tc.tile_pool(name="mask2", bufs=1) as mask_pool,
                tc.tile_pool(name="ost", bufs=2) as ost_pool,
                tc.tile_pool(name="qkv2", bufs=2) as qkv_pool,
                tc.tile_pool(name="expb", bufs=6) as exp_pool,
                tc.tile_pool(name="norm", bufs=3) as norm_pool,
                tc.tile_pool(name="ev", bufs=4) as ev_pool,
                tc.tile_pool(name="pss", bufs=3, space="PSUM") as ps_s,
                tc.tile_pool(name="pso", bufs=2, space="PSUM") as ps_o,
                tc.tile_pool(name="psc", bufs=1, space="PSUM") as ps_c,
                tc.tile_pool(name="psp", bufs=2, space="PSUM") as ps_p,
            ):
                mask_sb = mask_pool.tile([P, 4 * MC], BF16, tag="mask")
                nc.sync.dma_start(mask_sb[:], mask_d[:])
                wo_sb = wo_pool.tile([P, HPC * HS], BF16, tag="wo")
                wo_loaded = False

                def oproj(po, pcol, ct):
                    """One 128-col slice of the partial o-projection for the
                    query chunk whose normalized heads are in `po`."""
                    psp = ps_p.tile([P, MC], F32)
                    for hh in range(HPC):
                        nc.tensor.matmul(
                            psp[:],
                            wo_sb[:, hh * HS + ct * P: hh * HS + (ct + 1) * P],
                            po[hh][:],
                            start=(hh == 0), stop=(hh == HPC - 1),
                        )
                    ev = ev_pool.tile([P, MC], F32)
                    nc.any.tensor_copy(ev[:], psp[:])
                    nc.sync.dma_start(
                        out_d[ct * P:(ct + 1) * P, pcol:pcol + MC], ev[:])

                # o-projection of chunk N is emitted interleaved with the
                # attention of chunk N+1 (8 col-slices after each head) so
                # its matmuls fill the exp-wait bubbles on the PE
                pending = None
                for b in range(BS):
                    c0 = b * SL
                    qt, kt, vt = [], [], []
                    for h in range(HPC):
                        q_h = qkv_pool.tile([P, SL], BF16, tag=f"q{h}")
                        k_h = qkv_pool.tile([P, SL], BF16, tag=f"k{h}")
                        nc.sync.dma_start(
                            q_h[:], qT_d[h * P:(h + 1) * P, c0:c0 + SL])
                        nc.sync.dma_start(
                            k_h[:], kT_d[h * P:(h + 1) * P, c0:c0 + SL])
                        v_h = qkv_pool.tile([P, NJT * HD], BF16, tag=f"v{h}")
                        nc.sync.dma_start(
                            v_h[:].rearrange("p (j d) -> p j d", j=NJT),
                            v_d[c0:c0 + SL, h * HD:(h + 1) * HD]
                                .rearrange("(j p) d -> p j d", p=P),
                        )
                        qt.append(q_h); kt.append(k_h); vt.append(v_h)
                    if not wo_loaded:
                        # after the first head's q/k/v so the attention
                        # pipeline starts before this 4MB load
                        wo_loaded = True
                        nc.sync.dma_start(
                            wo_sb[:].rearrange("p (a c) -> p a c", a=HPC),
                            wo_d.rearrange("(a p) c -> p a c", p=P),
                        )
                    for ic in range(NIC):
                        njt = 4 * (ic + 1)
                        ost_ic = []
                        for h in range(HPC):
                            ps_out = ps_o.tile([P, MC], F32)
                            # exp tiles are pre-summed on the DVE in f32 so
                            # the partition-reduce needs ONE ones-matmul per
                            # (b, ic, h) instead of one per key tile; two
                            # interleaved chains halve the serial latency
                            es32 = norm_pool.tile([P, MC], F32, tag="es32")
                            es32b = None
                            if ic > 0:
                                es32b = norm_pool.tile(
                                    [P, MC], F32, tag="es32b", name="es32b")
                            # ic==0 is all-diagonal with partial widths, so
                            # it uses a single DVE chain (a partial-width
                            # init would leave garbage in the second chain)
                            dual = ic > 0
                            for jt in range(njt):
                                # diagonal tiles: queries left of the tile's
                                # first key are fully masked — skip those
                                # columns in score/exp/mask/AV entirely
                                t = jt - 4 * ic
                                col0 = t * P if t >= 0 else 0
                                W = MC - col0
                                ps_sc = ps_s.tile([P, MC], F32)
                                nc.tensor.matmul(
                                    ps_sc[:, col0:],
                                    kt[h][:, jt * P:(jt + 1) * P],
                                    qt[h][:, ic * MC + col0:(ic + 1) * MC],
                                    start=True, stop=True,
                                )
                                et = exp_pool.tile([P, MC], BF16)
                                nc.scalar.activation(
                                    et[:, col0:], ps_sc[:, col0:], ExpF)
                                if t >= 0:
                                    nc.vector.tensor_mul(
                                        et[:, col0:col0 + P],
                                        et[:, col0:col0 + P],
                                        mask_sb[:, t * MC + col0:
                                                t * MC + col0 + P])
                                nc.tensor.matmul(
                                    ps_out[:, col0:],
                                    vt[h][:, jt * HD:(jt + 1) * HD],
                                    et[:, col0:],
                                    start=(jt == 0), stop=(jt == njt - 1),
                                )
                                # two chains on two engines: DVE takes even
                                # key tiles, the otherwise-idle GpSimd odd
                                use_b = dual and jt % 2 == 1
                                eng = nc.gpsimd if use_b else nc.vector
                                acc = es32b if use_b else es32
                                init = jt == 0 or (dual and jt == 1)
                                if init:
                                    eng.tensor_copy(acc[:], et[:])
                                else:
                                    eng.tensor_add(
                                        acc[:, col0:], acc[:, col0:],
                                        et[:, col0:])
                            if dual:
                                nc.vector.tensor_add(
                                    es32[:], es32[:], es32b[:])
                            es16 = exp_pool.tile([P, MC], BF16, tag="es16")
                            nc.scalar.activation(es16[:], es32[:], CopyF)
                            ps_sum = ps_c.tile([P, MC], F32)
                            nc.tensor.matmul(
                                ps_sum[:], ones_sb[:], es16[:],
                                start=True, stop=True,
                            )
                            bcast = norm_pool.tile([P, MC], F32, tag="bcast")
                            nc.vector.reciprocal(bcast[:], ps_sum[:])
                            ost = ost_pool.tile([P, MC], BF16, tag=f"ost{h}",
                                                name=f"ost{h}")
                            nc.vector.tensor_mul(
                                ost[:], ps_out[:], bcast[:])
                            ost_ic.append(ost)
                            if pending is not None:
                                po, pcol = pending
                                for ct in range(h * 8, (h + 1) * 8):
                                    oproj(po, pcol, ct)
                        pending = (ost_ic, c0 + ic * MC)
                # flush the last chunk's o-projection
                po, pcol = pending
                for ct in range(HS // P):
                    oproj(po, pcol, ct)

    nc.compile()
    return nc


# Inputs that are identical on every core: shipped to the device ONCE
# (column-sharded) and replicated on-device by an all_gather program.
REPLICATED = ("xT", "cosq", "sinq", "cosk", "sink", "mask")


def _host_tables(hidden_states, wq, wk, wv, wo):
    """Single-copy replicated tables + per-core weight-slice stacks."""
    x = np.asarray(hidden_states, dtype=np.float32).reshape(ROWS, HS)
    xT = np.ascontiguousarray(x.T).astype(BFNP)

    inv_freq = 1.0 / (ROPE_THETA ** (np.arange(0, HD, 2, dtype=np.float32) / HD))
    pos = np.arange(SL, dtype=np.float32)
    freqs = pos[:, None] * inv_freq[None, :]
    emb = np.concatenate([freqs, freqs], axis=1)          # [SL, HD]
    cosT = np.cos(emb).astype(np.float32).T               # [HD, SL]
    sinT = np.sin(emb).astype(np.float32).T
    sign = np.ones((HD, 1), np.float32)
    sign[:HD // 2] = -1.0
    cosq = np.ascontiguousarray(np.tile(cosT, (1, BS)) * SCALE)
    sinq = np.ascontiguousarray(np.tile(sinT, (1, BS)) * sign * SCALE)
    cosk = np.ascontiguousarray(np.tile(cosT, (1, BS)))
    sink = np.ascontiguousarray(np.tile(sinT, (1, BS)) * sign)

    jj = np.arange(P)[:, None]
    ii = np.arange(MC)[None, :]
    mask = np.concatenate(
        [(t * P + jj <= ii) for t in range(4)], axis=1).astype(BFNP)

    wq = np.asarray(wq, np.float32)
    wk = np.asarray(wk, np.float32)
    wv = np.asarray(wv, np.float32)
    wo = np.asarray(wo, np.float32)

    # weight stacks: [N_CORES*rows, cols] so P("core") slices per core
    wq_s = np.concatenate(
        [wq[:, c * DPC:(c + 1) * DPC] for c in range(N_CORES)], 0).astype(BFNP)
    wk_s = np.concatenate(
        [wk[:, c * DPC:(c + 1) * DPC] for c in range(N_CORES)], 0).astype(BFNP)
    wv_s = np.concatenate(
        [wv[:, c * DPC:(c + 1) * DPC] for c in range(N_CORES)], 0).astype(BFNP)
    wo_s = np.ascontiguousarray(wo).astype(BFNP)   # [N_CORES*DPC, HS] already

    return (
        {"xT": xT, "cosq": cosq, "sinq": sinq, "cosk": cosk, "sink": sink,
         "mask": mask},
        {"wq": wq_s, "wk": wk_s, "wv": wv_s, "wo": wo_s},
    )


class Runner:
    """Compile the program once into a sharded PJRT executable; reuse across
    calls (no donation, so output buffers can stay device-resident)."""

    def __init__(self, nc):
        import jax
        import concourse.mybir as _mybir
        from concourse import bass2jax
        from jax.experimental.shard_map import shard_map
        from jax.sharding import Mesh, PartitionSpec, NamedSharding

        bass2jax.install_neuronx_cc_hook()
        self.jax = jax
        partition_name = (
            nc.partition_id_tensor.name if nc.partition_id_tensor else None)
        in_names, out_names, out_avals, zero_outs = [], [], [], []
        for alloc in nc.m.functions[0].allocations:
            if not isinstance(alloc, _mybir.MemoryLocationSet):
                continue
            name = alloc.memorylocations[0].name
            if alloc.kind == "ExternalInput":
                if name != partition_name:
                    in_names.append(name)
            elif alloc.kind == "ExternalOutput":
                shape = tuple(alloc.tensor_shape)
                dtype = _mybir.dt.np(alloc.dtype)
                out_names.append(name)
                out_avals.append(jax.core.ShapedArray(shape, dtype))
                zero_outs.append(np.zeros(shape, dtype))
        self.in_names, self.out_names = in_names, out_names
        self.out_avals = out_avals
        all_names = list(in_names)
        if partition_name is not None:
            all_names = all_names + [partition_name]

        def _body(*args):
            operands = list(args)
            if partition_name is not None:
                operands.append(bass2jax.partition_id_tensor())
            outs = bass2jax._bass_exec_p.bind(
                *operands,
                out_avals=tuple(out_avals),
                in_names=tuple(all_names),
                out_names=tuple(out_names),
                lowering_input_output_aliases=(),
                sim_require_finite=True,
                sim_require_nnan=True,
                nc=nc,
            )
            return tuple(outs)

        devices = jax.devices()[:N_CORES]
        mesh = Mesh(np.asarray(devices), ("core",))
        self.mesh = mesh
        self.sharding = NamedSharding(mesh, PartitionSpec("core"))
        self.fn = jax.jit(
            shard_map(
                _body, mesh=mesh,
                in_specs=(PartitionSpec("core"),) * len(in_names),
                out_specs=(PartitionSpec("core"),) * len(out_names),
                check_rep=False,
            ),
            keep_unused=True,
        )

        # pre: on-device replication of shared tables (ship 1/8 of the bytes,
        # all_gather to a per-core full copy stacked on axis 0)
        col_sharding = NamedSharding(mesh, PartitionSpec(None, "core"))
        self.col_sharding = col_sharding

        def _pre_body(*cols):
            return tuple(
                jax.lax.all_gather(c, "core", axis=1, tiled=True)
                for c in cols)

        n_rep = len(REPLICATED)
        self.pre = jax.jit(
            shard_map(
                _pre_body, mesh=mesh,
                in_specs=(PartitionSpec(None, "core"),) * n_rep,
                out_specs=(PartitionSpec("core", None),) * n_rep,
                check_rep=False,
            ))

        # post: on-device 8-way reduction of the partial o-projections plus
        # transpose into the final [ROWS, HS] layout
        import jax.numpy as jnp

        def _post_body(p):
            r = jax.lax.psum_scatter(p, "core", scatter_dimension=0,
                                     tiled=True)     # [DPC, ROWS] f32
            return jnp.transpose(r)                  # [ROWS, DPC]

        self.post = jax.jit(
            shard_map(
                _post_body, mesh=mesh,
                in_specs=(PartitionSpec("core", None),),
                out_specs=PartitionSpec(None, "core"),
                check_rep=False,
            ))

    def prepare_device_args(self, tables, weights):
        """Ship inputs to the cores: shared tables column-sharded then
        all-gathered on device; weight stacks row-sharded directly."""
        jax = self.jax
        rep_dev = self.pre(*[
            jax.device_put(tables[n], self.col_sharding) for n in REPLICATED])
        by_name = dict(zip(REPLICATED, rep_dev))
        for n, w in weights.items():
            by_name[n] = jax.device_put(w, self.sharding)
        return [by_name[n] for n in self.in_names]

    def finalize(self, out_arrs):
        """Reduce the 8 partial o-projections on device; fetch [ROWS, HS]."""
        return np.asarray(self.post(out_arrs[0]))


_RUNNER = None


def get_runner():
    global _RUNNER
    if _RUNNER is None:
        _RUNNER = Runner(build_program())
    return _RUNNER


def kernel(hidden_states, wq, wk, wv, wo):
    runner = get_runner()
    tables, weights = _host_tables(hidden_states, wq, wk, wv, wo)
    dev_args = runner.prepare_device_args(tables, weights)
    out_arrs = runner.fn(*dev_args)
    return runner.finalize(out_arrs).reshape(BS, SL, HS)

